# revision 20
# baseline (speedup 1.0000x reference)
"""Cross-modal attention (B=4, C=512, L=2048, H=8, D=64) on 8 TRN2 NeuronCores.

Sharding: core c handles batch b = c//2 and query-half q = c%2 (1024 queries).
K/V are computed from the full ecg[b] on both cores of a pair.

v2 design: the softmax exp on the ACT engine is the hard floor (~131us of
exp work per core), so the kernel is organized to keep ACT 100% busy and
hide everything else under it:
  - all projection / context / output matmuls run as fp8e4 DoubleRow
    (2 contraction tiles per instruction = 2x bf16 throughput, verified
    on HW at ~266ns per 256x128x512 instruction)
  - scores stay bf16 (contract=64 gets no DoubleRow benefit)
  - phase-1 (K/Q/V projections) and phase-3 (output projection) matmuls
    are interleaved into the PE stream as "fillers" between score/ctx
    matmuls so the PE never idles (keeps the HAM p-state at 2.4GHz)
  - exp reads 2 key-blocks of scores from PSUM per instruction and writes
    fp8e4 directly; the context DoubleRow consumes both blocks at once.

Layouts: inputs arrive as (C, L) = x^T which is already the lhsT/rhs layout
the TensorEngine wants; no runtime transposes anywhere.  The softmax
denominator rides along as a ones-column in v (row 64 of each ctx psum).
"""

import os
from collections import deque

import numpy as np

B = 4
C = 512
L = 2048
H = 8
D = 64
LQ = 1024          # queries per core
P = 128
NCB = C // P       # 4 channel blocks
NKBP = 8           # key-block pairs (256 keys each)

_CACHED = {}


def _build():
    import concourse.tile as tile
    from concourse import bacc, mybir

    F32 = mybir.dt.float32
    BF16 = mybir.dt.bfloat16
    FP8 = mybir.dt.float8e4
    DR = mybir.MatmulPerfMode.DoubleRow
    EXP = mybir.ActivationFunctionType.Exp

    nc = bacc.Bacc("TRN2", target_bir_lowering=False, debug=False)

    ppg_q = nc.dram_tensor("ppg_q", (C, LQ), F32, kind="ExternalInput").ap()
    ecg_b = nc.dram_tensor("ecg_b", (C, L), F32, kind="ExternalInput").ap()
    # host-prepacked DR layouts: (p, s2*cb*j*m) / (p, s2*lb*j*m)
    ecg_v = nc.dram_tensor("ecg_v", (P, 2 * 16 * 2 * P), F32,
                           kind="ExternalInput").ap()
    wqt = nc.dram_tensor("wqt", (P, 2 * NCB * 2 * P), F32,
                         kind="ExternalInput").ap()
    wkt = nc.dram_tensor("wkt", (P, 2 * NCB * 2 * P), F32,
                         kind="ExternalInput").ap()
    wvt = nc.dram_tensor("wvt", (C, C), F32, kind="ExternalInput").ap()
    wot = nc.dram_tensor("wot", (P, 2 * NCB * 2 * P), F32,
                         kind="ExternalInput").ap()
    bq = nc.dram_tensor("bq", (C,), F32, kind="ExternalInput").ap()
    bk = nc.dram_tensor("bk", (C,), F32, kind="ExternalInput").ap()
    bv = nc.dram_tensor("bv", (C,), F32, kind="ExternalInput").ap()
    bo = nc.dram_tensor("bo", (C,), F32, kind="ExternalInput").ap()
    outp = nc.dram_tensor("outp", (C, LQ), F32, kind="ExternalOutput").ap()
    dbg = {}
    if os.environ.get("KDBG"):
        for name, shape in [("qT", (P, NCB, LQ)), ("kT", (P, NCB, L)),
                            ("v8", (P, NKBP, H, 2, D + 16)),
                            ("ctxT", (P, NCB, LQ)),
                            ("et0", (P, 2, 512))]:
            dbg[name] = nc.dram_tensor("d_" + name, shape, F32,
                                       kind="ExternalOutput").ap()

    with tile.TileContext(nc) as tc:
        with (
            tc.tile_pool(name="persist", bufs=1) as persist,
            tc.tile_pool(name="pp_ps", bufs=1, space="PSUM") as pp_ps,
            tc.tile_pool(name="st_ps", bufs=1, space="PSUM") as st_ps,
            tc.tile_pool(name="pc_ps", bufs=1, space="PSUM") as pc_ps,
            tc.tile_pool(name="et_pool", bufs=1) as et_pool,
            tc.tile_pool(name="sm_pool", bufs=2) as sm_pool,
            tc.tile_pool(name="out_sb", bufs=3) as out_sb,
        ):
            # ---- persistent tiles ----
            # lhsT weights in DoubleRow-paired layout: [p, s2, cb, j, m] so
            # each DR weight tile [2, 128] is contiguous (ISA requirement)
            wq8 = persist.tile([P, 2, NCB, 2, P], FP8)
            wk8 = persist.tile([P, 2, NCB, 2, P], FP8)
            wo8 = persist.tile([P, 2, NCB, 2, P], FP8)
            wv8m = persist.tile([P, NCB, C], FP8)       # moving operand
            ecg8 = persist.tile([P, NCB, L], FP8)       # moving (k-proj rhs)
            ecg8v = persist.tile([P, 2, L // P, 2, P], FP8)  # lhsT (v-proj)
            ppg8 = persist.tile([P, NCB, LQ], FP8)
            ppg_f = persist.tile([P, NCB, LQ], F32)
            bq_t = persist.tile([P, NCB], F32)
            bk_t = persist.tile([P, NCB], F32)
            bo_t = persist.tile([P, NCB], F32)
            bv_row = persist.tile([1, C], BF16)
            ones = persist.tile([1, P], BF16)
            oc8 = persist.tile([P, 1], FP8)
            qT = persist.tile([P, NCB, LQ], BF16)
            kT = persist.tile([P, NCB, L], BF16)
            # v in DR-lhsT layout: [p, kbp, h, j, 80]; col 64 = ones (softmax
            # denominator), cols 65-79 pad (dual-fp8 ldweights needs
            # column count % 16 == 0)
            v8 = persist.tile([P, NKBP, H, 2, D + 16], FP8)
            ctxT = persist.tile([P, NCB, LQ], FP8)

            # ---- input DMAs, ordered by first need ----
            ecg_r = ecg_b.rearrange("(s p) l -> p s l", p=P)
            w_dr = "p (s2 cb j m) -> p s2 cb j m"
            ev_dr = "p (s2 lb j m) -> p s2 lb j m"
            nc.gpsimd.dma_start(wk8[:], wkt.rearrange(w_dr, cb=NCB, j=2, m=P))
            nc.gpsimd.dma_start(ecg8[:, :, 0:512], ecg_r[:, :, 0:512])
            nc.gpsimd.dma_start(
                ecg8v[:, :, 0:4, :, :],
                ecg_v.rearrange(ev_dr, lb=16, j=2, m=P)[:, :, 0:4, :, :])
            nc.gpsimd.dma_start(bv_row[0:1, :], bv[None, :])
            nc.gpsimd.dma_start(wq8[:], wqt.rearrange(w_dr, cb=NCB, j=2, m=P))
            nc.gpsimd.dma_start(ppg8[:], ppg_q.rearrange("(s p) l -> p s l", p=P))
            nc.gpsimd.dma_start(wv8m[:], wvt.rearrange("(s p) o -> p s o", p=P))
            for kc in range(1, 4):
                nc.gpsimd.dma_start(ecg8[:, :, kc * 512:(kc + 1) * 512],
                                    ecg_r[:, :, kc * 512:(kc + 1) * 512])
                nc.gpsimd.dma_start(
                    ecg8v[:, :, kc * 4:(kc + 1) * 4, :, :],
                    ecg_v.rearrange(ev_dr, lb=16, j=2,
                                    m=P)[:, :, kc * 4:(kc + 1) * 4, :, :])
            nc.gpsimd.dma_start(wo8[:], wot.rearrange(w_dr, cb=NCB, j=2, m=P))
            nc.sync.dma_start(bq_t[:], bq.rearrange("(s p) -> p s", p=P))
            nc.sync.dma_start(bk_t[:], bk.rearrange("(s p) -> p s", p=P))
            nc.sync.dma_start(bo_t[:], bo.rearrange("(s p) -> p s", p=P))
            nc.sync.dma_start(ppg_f[:], ppg_q.rearrange("(s p) l -> p s l", p=P))
            nc.vector.memset(ones[:], 1.0)
            nc.vector.memset(oc8[:], 1.0)
            nc.vector.tensor_copy(
                out=v8[:, :, :, :, D:D + 16],
                in_=oc8[:, None, None, None, :].to_broadcast((P, NKBP, H, 2, 16)))

            # ---- phase-1 / phase-3 task definitions ----
            def k_proj(cb, kc):
                ps = pp_ps.tile([P, 512], F32, tag="pp", bufs=2)
                for s2 in (0, 1):
                    nc.tensor.matmul(
                        ps[:], wk8[:, s2, cb, :, :],
                        ecg8[:, 2 * s2:2 * s2 + 2, kc * 512:(kc + 1) * 512],
                        start=(s2 == 0), stop=(s2 == 1), perf_mode=DR)
                nc.vector.tensor_scalar_add(
                    kT[:, cb, kc * 512:(kc + 1) * 512], ps[:],
                    bk_t[:, cb:cb + 1])

            def q_proj(cb, qb):
                ps = pp_ps.tile([P, 512], F32, tag="pp", bufs=2)
                for s2 in (0, 1):
                    nc.tensor.matmul(
                        ps[:], wq8[:, s2, cb, :, :],
                        ppg8[:, 2 * s2:2 * s2 + 2, qb * 512:(qb + 1) * 512],
                        start=(s2 == 0), stop=(s2 == 1), perf_mode=DR)
                nc.vector.tensor_scalar_add(
                    qT[:, cb, qb * 512:(qb + 1) * 512], ps[:],
                    bq_t[:, cb:cb + 1])

            def v_proj(lb):
                ps = pp_ps.tile([P, 512], F32, tag="pp", bufs=2)
                nc.tensor.matmul(ps[:], ones[0:1, :], bv_row[0:1, :],
                                 start=True, stop=False)
                for s2 in (0, 1):
                    nc.tensor.matmul(
                        ps[:], ecg8v[:, s2, lb, :, :],
                        wv8m[:, 2 * s2:2 * s2 + 2, :],
                        start=False, stop=(s2 == 1), perf_mode=DR)
                nc.vector.tensor_copy(
                    out=v8[:, lb // 2, :, lb % 2, 0:D],
                    in_=ps[:].rearrange("p (h d) -> p h d", d=D))

            def o_proj(cb, qb):
                qsl = slice(qb * 512, (qb + 1) * 512)
                ps = pp_ps.tile([P, 512], F32, tag="pp", bufs=2)
                for j2 in (0, 1):
                    nc.tensor.matmul(
                        ps[:], wo8[:, j2, cb, :, :],
                        ctxT[:, 2 * j2:2 * j2 + 2, qsl],
                        start=(j2 == 0), stop=(j2 == 1), perf_mode=DR)
                ot = out_sb.tile([P, 512], F32)
                nc.vector.tensor_scalar_add(ot[:], ps[:], bo_t[:, cb:cb + 1])
                nc.vector.tensor_add(ot[:], ot[:], ppg_f[:, cb, qsl])
                nc.sync.dma_start(
                    outp.rearrange("(s p) l -> p s l", p=P)[:, cb, qsl], ot[:])

            # ---- deadline-driven task emission ----
            # every phase-1/3 task keyed; require() emits a specific task
            # just-in-time (before its consumer lands in PE program order),
            # drain() soaks one pending task per kbp slot to pull work early
            tasks = {}
            order = deque()

            def add_task(key, fn):
                tasks[key] = fn
                order.append(key)

            for lb in range(16):
                add_task(("v", lb), lambda lb=lb: v_proj(lb))
            for cb in range(4):
                for kc in range(4):
                    add_task(("k", cb, kc), lambda cb=cb, kc=kc: k_proj(cb, kc))
            for cb in range(4):
                add_task(("q", cb, 0), lambda cb=cb: q_proj(cb, 0))
            for cb in range(4):
                add_task(("q", cb, 1), lambda cb=cb: q_proj(cb, 1))

            def require(key):
                fn = tasks.pop(key, None)
                if fn is not None:
                    fn()
                    return True
                return False

            def drain(n=1):
                done = 0
                while done < n and order:
                    key = order.popleft()
                    if require(key):
                        done += 1

            def pe_warm(n):
                # dummy weight loads: keep the PE busy-streak alive through
                # ACT-paced dependency gaps so the clock stays at 2.4GHz
                for _ in range(n):
                    nc.tensor.ldweights(ones[0:1, 0:16])

            # ---- main attention loop ----
            for qb in range(2):
                qsl = slice(qb * 512, (qb + 1) * 512)
                for pair in range(4):
                    for hl in range(2):
                        h = 2 * pair + hl
                        hb = 64 * hl
                        pc = pc_ps.tile([P, 512], F32, tag="pc", bufs=2)
                        for kbp in range(NKBP):
                            req = require(("q", pair, qb)) if kbp == 0 else False
                            req |= require(("k", pair, kbp // 2))
                            req |= require(("v", 2 * kbp))
                            req |= require(("v", 2 * kbp + 1))
                            if not req:
                                drain(1)
                            st = st_ps.tile([P, 2, 512], F32, tag="st", bufs=2)
                            for j in range(2):
                                kb = 2 * kbp + j
                                nc.tensor.matmul(
                                    st[:, j, :],
                                    kT[hb:hb + 64, pair, kb * P:(kb + 1) * P],
                                    qT[hb:hb + 64, pair, qsl],
                                    start=True, stop=True)
                            et = et_pool.tile([P, 2, 512], FP8, tag="et",
                                              bufs=4)
                            nc.scalar.activation(et[:], st[:], EXP, scale=0.125)
                            if dbg and qb == 0 and pair == 0 and hl == 0 \
                                    and kbp == 0:
                                nc.gpsimd.dma_start(dbg["et0"], et[:])
                            nc.tensor.matmul(
                                pc[0:D + 16, :], v8[:, kbp, h, :, :], et[:],
                                start=(kbp == 0), stop=(kbp == NKBP - 1),
                                perf_mode=DR)
                            pe_warm(6)
                        # softmax normalization (off the ACT critical path)
                        den = sm_pool.tile([1, 512], F32)
                        nc.vector.tensor_copy(out=den[0:1, :],
                                              in_=pc[D:D + 1, :])
                        recip = sm_pool.tile([1, 512], F32)
                        nc.vector.reciprocal_approx_fast(out=recip[0:1, :],
                                                         in_=den[0:1, :])
                        rbc = sm_pool.tile([64, 512], F32)
                        nc.gpsimd.partition_broadcast(rbc[:], recip[0:1, :],
                                                      channels=64)
                        nc.vector.tensor_mul(out=ctxT[hb:hb + 64, pair, qsl],
                                             in0=pc[0:D, :], in1=rbc[:])
                        pe_warm(8)
                # phase 3 for this query half; qb=0 fills qb=1's regions
                if qb == 0:
                    for cb in range(4):
                        add_task(("o", cb, 0), lambda cb=cb: o_proj(cb, 0))
                else:
                    drain(len(order))
                    for cb in range(4):
                        o_proj(cb, 1)
            if dbg:
                for name, src in (("qT", qT), ("kT", kT), ("v8", v8),
                                  ("ctxT", ctxT)):
                    nc.gpsimd.dma_start(dbg[name], src[:])
    nc.compile()
    return nc


def _get_nc():
    if "nc" not in _CACHED:
        _CACHED["nc"] = _build()
    return _CACHED["nc"]


def kernel(ppg, ecg, Wq, bq, Wk, bk, Wv, bv, Wo, bo):
    from concourse.bass_utils import run_bass_kernel_spmd

    nc = _get_nc()
    f = np.float32

    def w_pack(w):
        # W.T (C,C) -> (p, s2, cb, j, m) flat: DR-paired lhsT layout
        wt = np.asarray(w, f).T
        return np.ascontiguousarray(
            wt.reshape(2, 2, P, NCB, P).transpose(2, 0, 3, 1, 4).reshape(
                P, 2 * NCB * 2 * P))

    def e_pack(e):
        # ecg (C,L) -> (p, s2, lb, j, m) flat: DR-paired lhsT layout
        return np.ascontiguousarray(
            e.reshape(2, 2, P, 16, P).transpose(2, 0, 3, 1, 4).reshape(
                P, 2 * 16 * 2 * P))

    wqt = w_pack(Wq)
    wkt = w_pack(Wk)
    wot = w_pack(Wo)
    wvt = np.ascontiguousarray(np.asarray(Wv, f).T)
    ppg = np.asarray(ppg, f)
    ecg = np.asarray(ecg, f)
    in_maps = []
    for c in range(8):
        b, half = c // 2, c % 2
        in_maps.append({
            "ppg_q": np.ascontiguousarray(ppg[b][:, half * LQ:(half + 1) * LQ]),
            "ecg_b": np.ascontiguousarray(ecg[b]),
            "ecg_v": e_pack(ecg[b]),
            "wqt": wqt, "wkt": wkt, "wvt": wvt, "wot": wot,
            "bq": np.asarray(bq, f), "bk": np.asarray(bk, f),
            "bv": np.asarray(bv, f), "bo": np.asarray(bo, f),
        })
    _CACHED["last_in_maps"] = in_maps
    res = run_bass_kernel_spmd(nc, in_maps, core_ids=list(range(8)))
    out = np.empty((B, C, L), f)
    for c, r in enumerate(res.results):
        b, half = c // 2, c % 2
        out[b][:, half * LQ:(half + 1) * LQ] = r["outp"]
    return out
